# revision 29
# baseline (speedup 1.0000x reference)
"""Trainium2 Bass kernel for nn_Cross_Attention (B=8, N=2048, D=768).

Math (per batch b):
    A   = softmax(t, axis=-1) = E/R     (E = exp(t), R = rowsum)
    Q   = softmax(t, axis=0)  = E/S     (S = colsum)
    attn = (x @ A^T) @ Q = x @ KQ,   KQ[d,d'] = (sum_m E[m,d] E[m,d']/R[m]) / S[d']
    out = x @ Msum + fb + x
    Msum = f*(KQ_1 @ W1^T + KQ_2 @ W2^T),  fb = f*(b1 + b2),  f = sigmoid(w)

All heavy matmuls are fp8e4 with MatmulPerfMode.DoubleRow (2 k-tiles
per instruction, 2x PE rate).  The 1/R row normalization rides the
*stationary* operand only (sc = E * C1/R), so KQ Gram matmuls for a
token pair depend only on that pair's exp + rowsum.  The 1/S column
normalization is folded into the *weights* (wts *= 4096/S, in place),
which makes kqt = KQraw^T / 16 symmetric with a constant copy-out
scale - no per-column rescale, no symmetric-fill pass.

Schedule:
  - x2/x3/x arrive as bf16 (host-cast): halves input HBM traffic.
  - x^T comes from 6 XBAR DMA-transposes (one per 128-feature block,
    [2048,128] DRAM -> [128,2048] SBUF), then one fp8 cast per block:
    zero PE/copy cost for transposition.
  - streamed per 2-tile pair (per t): exp (ACT, accum R) -> 1/R (DVE)
    -> sc (gpsimd) -> colsum + KQ-dp0 row (PE, DoubleRow).
  - KQ dp1-5 full rows don't fit PSUM next to both colsum accumulators:
    they run as two PE bursts per t right after that t's stream,
    overlapping the other t's stream / B / C.
  - B (Msum) uses the S-scaled weights; C streams y = xt8 @ msum
    (+ fb via a tiny bf16 ones-matmul) and adds the exact bf16 residual.

Scales: g=E; sc=E*1024/R; kqt=KQraw^T/16; wts=32*f*W^T*4096/S;
msum=(sum kqt*wts)/8 = 1024*Msum; xt8=8*x^T;
y_ps = 8192*(x@Msum+fb); out = y_ps/8192 + x.
"""

import numpy as np
import ml_dtypes

import concourse.bass as bass
import concourse.tile as tile
from concourse import bacc
from concourse import mybir
from concourse.bass_utils import run_bass_kernel_spmd

F32 = mybir.dt.float32
BF16 = mybir.dt.bfloat16
FP8 = mybir.dt.float8e4
DR = mybir.MatmulPerfMode.DoubleRow
MUL = mybir.AluOpType.mult

B = 8
P = 128
D = 768
DT = D // P
C1 = 1024.0
CS = 1.0 / 16.0       # kqt = CS * KQraw^T;  copy scale = CS/C1
CSW = 4096.0          # wts *= CSW/S
CM_DIV = 8.0
CY = 8.0
Y_SCALE = 1.0 / 8192.0

CHUNKS = ((0, 512), (512, 256))

# ---- PSUM map (f32 offsets; banks are 512 f32) ----
# stream: dp0[t0]@0, s[t0]@1024, dp0[t1]@2048, s[t1]@3072, tp@3840/3968
# post bursts reuse freed stream regions (see POST_BURSTS)
# B: m_ps@0/1024;  C: y_ps@0/1024
DP0_OFF = (0, 2048)
S_OFF = (1024, 3072)
TP_OFF = (3840, 3968)
# per burst: list of (dp, ((col_off, width, psum_off), ...))
POST_BURSTS = (
    ((1, ((0, 512, 0), (512, 256, 512))),
     (2, ((0, 512, 1024), (512, 256, 768))),
     (3, ((0, 512, 1536), (512, 256, 1792)))),
    ((4, ((0, 512, 0), (512, 256, 512))),
     (5, ((0, 512, 1024), (512, 256, 768)))),
)


def build_nc(NT=16):
    N = NT * P
    NP = NT // 2
    nc = bacc.Bacc()

    x_d = nc.dram_tensor("x", [N, D], BF16, kind="ExternalInput")
    x2_d = nc.dram_tensor("x2", [N, D], BF16, kind="ExternalInput")
    x3_d = nc.dram_tensor("x3", [N, D], BF16, kind="ExternalInput")
    wt1_d = nc.dram_tensor("wt1", [D, D], FP8, kind="ExternalInput")
    wt2_d = nc.dram_tensor("wt2", [D, D], FP8, kind="ExternalInput")
    fb_d = nc.dram_tensor("fb", [1, D], F32, kind="ExternalInput")
    id_d = nc.dram_tensor("ident", [P, P], F32, kind="ExternalInput")
    out_d = nc.dram_tensor("out", [N, D], BF16, kind="ExternalOutput")

    x2_pr = x2_d.rearrange("(q t p) d -> q p t d", t=2, p=P)
    x3_pr = x3_d.rearrange("(q t p) d -> q p t d", t=2, p=P)
    x_half = x_d.rearrange("(c t p) d -> c p t d", t=8, p=P)
    att_pr = [x2_pr, x3_pr]
    out_t = out_d.rearrange("(t p) d -> t p d", p=P)

    with tile.TileContext(nc) as tc:
        with (
            tc.tile_pool(name="ps", bufs=1, space="PSUM") as psp,
            tc.tile_pool(name="consts", bufs=1) as consts,
            tc.tile_pool(name="big", bufs=1) as big,
            tc.tile_pool(name="stream", bufs=2) as stream,
            tc.tile_pool(name="stats", bufs=2) as stats,
            tc.tile_pool(name="outp", bufs=3) as outp,
        ):
            psb = psp.tile([P, 4096], F32)

            ones8 = consts.tile([P, 2, P], FP8)
            nc.vector.memset(ones8, 1.0)
            onesfb = consts.tile([P, P], BF16)
            nc.vector.memset(onesfb, 1.0 / 128.0)
            ident = consts.tile([P, P], F32)
            fbb = consts.tile([P, D], F32)
            fbby = consts.tile([P, D], BF16)
            wts = consts.tile([P, 2, DT, D], FP8)
            kqt = [
                consts.tile([P, DT, D], FP8, tag=f"kqt{t}", name=f"kqt{t}")
                for t in range(2)
            ]
            msum = consts.tile([P, DT, D], FP8)
            rsw = consts.tile([P, 2, DT], F32)  # CSW/S per d'
            xbig = consts.tile([P, NT, D], BF16)
            xtb = consts.tile([P, DT, N], BF16)  # x^T via XBAR
            xt8 = consts.tile([P, DT, N], FP8)   # 8*x^T
            gate = consts.tile([P, 1], BF16)
            g = [
                big.tile([P, NT, D], FP8, tag=f"g{t}", name=f"g{t}")
                for t in range(2)
            ]
            sc = [
                big.tile([P, NT, D], FP8, tag=f"sc{t}", name=f"sc{t}")
                for t in range(2)
            ]
            xsrc = [
                big.tile([P, NT, D], BF16, tag=f"xs{t}", name=f"xs{t}")
                for t in range(2)
            ]

            # ---- all input DMAs up-front; rings serialize the transfers ----
            # sync ring: x2-even pairs, x3-even pairs, x^T XBAR blocks
            # scalar ring: x2-odd pairs, x3-odd pairs, x residual halves
            for q in range(0, NP, 2):
                nc.sync.dma_start(out=xsrc[0][:, 2 * q : 2 * q + 2, :],
                                  in_=x2_pr[q])
            for q in range(1, NP, 2):
                nc.scalar.dma_start(out=xsrc[0][:, 2 * q : 2 * q + 2, :],
                                    in_=x2_pr[q])
            for q in range(0, NP, 2):
                nc.sync.dma_start(out=xsrc[1][:, 2 * q : 2 * q + 2, :],
                                  in_=x3_pr[q])
            for q in range(1, NP, 2):
                nc.scalar.dma_start(out=xsrc[1][:, 2 * q : 2 * q + 2, :],
                                    in_=x3_pr[q])
            for c in range(DT):
                nc.sync.dma_start_transpose(
                    xtb[:, c, :], x_d[:, c * P : (c + 1) * P]
                )
            for h in range(2):
                nc.scalar.dma_start(out=xbig[:, 8 * h : 8 * h + 8, :],
                                    in_=x_half[h])

            # gpsimd consts, gated behind the first input pair
            nc.gpsimd.dma_start(out=gate, in_=xsrc[0][:, 0, 0:1])
            for tw, wd in enumerate((wt1_d, wt2_d)):
                nc.gpsimd.dma_start(
                    out=wts[:, tw], in_=wd.rearrange("(c p) j -> p c j", p=P)
                )
            nc.gpsimd.dma_start(out=ident, in_=id_d[:, :])
            nc.gpsimd.dma_start(out=fbb, in_=fb_d[0:1, :].to_broadcast([P, D]))
            nc.vector.tensor_copy(fbby, fbb)

            rvec = stats.tile([P, 2, NT], F32, tag="rvec")
            rrec = stats.tile([P, 2, NT], F32, tag="rrec")

            def stream_t(t):
                """exp + 1/R + sc + colsum + KQ dp0 row, per pair."""
                for q in range(NP):
                    for j in range(2):
                        i = 2 * q + j
                        nc.scalar.activation(
                            out=g[t][:, i, :], in_=xsrc[t][:, i, :],
                            func=mybir.ActivationFunctionType.Exp,
                            accum_out=rvec[:, t, i : i + 1],
                        )
                    nc.vector.reciprocal(rrec[:, t, 2 * q : 2 * q + 2],
                                         rvec[:, t, 2 * q : 2 * q + 2])
                    for j in range(2):
                        i = 2 * q + j
                        nc.gpsimd.tensor_scalar(
                            out=sc[t][:, i, :], in0=g[t][:, i, :],
                            scalar1=rrec[:, t, i : i + 1], scalar2=C1,
                            op0=MUL, op1=MUL,
                        )
                    gpr = g[t][:, 2 * q : 2 * q + 2, :]
                    spr = sc[t][:, 2 * q : 2 * q + 2, :]
                    s_ps = psb[:, S_OFF[t] : S_OFF[t] + D]
                    d0 = psb[:, DP0_OFF[t] : DP0_OFF[t] + D]
                    for off, sz in CHUNKS:
                        nc.tensor.matmul(
                            s_ps[:, off : off + sz], ones8,
                            gpr[:, :, off : off + sz],
                            start=(q == 0), stop=(q == NP - 1), perf_mode=DR,
                        )
                    for off, sz in CHUNKS:
                        nc.tensor.matmul(
                            d0[:, off : off + sz], spr[:, :, 0:P],
                            gpr[:, :, off : off + sz],
                            start=(q == 0), stop=(q == NP - 1), perf_mode=DR,
                        )

            def epilogue_t(t):
                """dp0 copy, 4096/S, wts scaling, dp1-5 bursts."""
                nc.vector.tensor_scalar_mul(
                    kqt[t][:, 0, :], psb[:, DP0_OFF[t] : DP0_OFF[t] + D],
                    CS / C1,
                )
                s_ps = psb[:, S_OFF[t] : S_OFF[t] + D]
                rsb = stream.tile([P, D], F32, tag="rsb", bufs=2)
                nc.vector.reciprocal(rsb, s_ps)
                nc.vector.tensor_scalar_mul(rsb, rsb, CSW)
                for c in range(DT):
                    tp = psb[:, TP_OFF[c % 2] : TP_OFF[c % 2] + P]
                    nc.tensor.transpose(tp, rsb[:, c * P : (c + 1) * P], ident)
                    nc.vector.tensor_copy(rsw[:, t, c : c + 1], tp[:, 0:1])
                for dp in range(DT):
                    # fold 1/S into the weights (in place, per d' row)
                    nc.vector.tensor_scalar_mul(
                        wts[:, t, dp, :], wts[:, t, dp, :],
                        rsw[:, t, dp : dp + 1],
                    )
                for burst in POST_BURSTS:
                    for dp, chunks in burst:
                        lhsT_all = sc[t]
                        for off, sz, poff in chunks:
                            for q in range(NP):
                                nc.tensor.matmul(
                                    psb[:, poff : poff + sz],
                                    sc[t][:, 2 * q : 2 * q + 2,
                                          dp * P : (dp + 1) * P],
                                    g[t][:, 2 * q : 2 * q + 2, off : off + sz],
                                    start=(q == 0), stop=(q == NP - 1),
                                    perf_mode=DR,
                                )
                    for dp, chunks in burst:
                        for off, sz, poff in chunks:
                            nc.vector.tensor_scalar_mul(
                                kqt[t][:, dp, off : off + sz],
                                psb[:, poff : poff + sz],
                                CS / C1,
                            )

            stream_t(0)
            epilogue_t(0)
            # 8*x^T fp8 cast (xtb blocks arrive during t1's stream)
            for c in range(DT):
                nc.vector.tensor_scalar_mul(xt8[:, c, :], xtb[:, c, :], CY)
            stream_t(1)
            epilogue_t(1)

            # ---- Msum ----
            for d in range(DT):
                mb = 1024 * (d % 2)
                m_ps = psb[:, mb : mb + D]
                for t in range(2):
                    for dpp in range(0, DT, 2):
                        lhsT = kqt[t][:, dpp : dpp + 2, d * P : (d + 1) * P]
                        for off, sz in CHUNKS:
                            nc.tensor.matmul(
                                m_ps[:, off : off + sz], lhsT,
                                wts[:, t, dpp : dpp + 2, off : off + sz],
                                start=(t == 0 and dpp == 0),
                                stop=(t == 1 and dpp == DT - 2),
                                perf_mode=DR,
                            )
                if d % 2 == 0:
                    nc.vector.tensor_scalar_mul(msum[:, d, :], m_ps, 1.0 / CM_DIV)
                else:
                    nc.scalar.mul(msum[:, d, :], m_ps, 1.0 / CM_DIV)

            # ---- y phase ----
            for i in range(NT):
                yb = 1024 * (i % 2)
                y_ps = psb[:, yb : yb + D]
                for off, sz in CHUNKS:
                    nc.tensor.matmul(
                        y_ps[:, off : off + sz], onesfb, fbby[:, off : off + sz],
                        start=True, stop=False,
                    )
                for k in range(0, DT, 2):
                    for off, sz in CHUNKS:
                        nc.tensor.matmul(
                            y_ps[:, off : off + sz],
                            xt8[:, k : k + 2, i * P : (i + 1) * P],
                            msum[:, k : k + 2, off : off + sz],
                            start=False, stop=(k == DT - 2),
                            perf_mode=DR,
                        )
                oi = outp.tile([P, D], F32, tag="out")
                nc.vector.scalar_tensor_tensor(
                    out=oi, in0=y_ps, scalar=Y_SCALE, in1=xbig[:, i, :],
                    op0=MUL, op1=mybir.AluOpType.add,
                )
                nc.gpsimd.dma_start(out=out_t[i], in_=oi)

    nc.compile()
    return nc


def prep_inputs(inputs):
    x = np.asarray(inputs["x"], dtype=np.float32)
    x2 = np.asarray(inputs["x2"], dtype=np.float32)
    x3 = np.asarray(inputs["x3"], dtype=np.float32)
    W1 = np.asarray(inputs["W1"], dtype=np.float32)
    b1 = np.asarray(inputs["b1"], dtype=np.float32)
    W2 = np.asarray(inputs["W2"], dtype=np.float32)
    b2 = np.asarray(inputs["b2"], dtype=np.float32)
    w = np.asarray(inputs["w"], dtype=np.float32)

    xb = np.ascontiguousarray(x).astype(ml_dtypes.bfloat16)
    x2b = np.ascontiguousarray(x2).astype(ml_dtypes.bfloat16)
    x3b = np.ascontiguousarray(x3).astype(ml_dtypes.bfloat16)

    f = 1.0 / (1.0 + np.exp(-float(w.reshape(-1)[0])))
    wt1 = np.ascontiguousarray((32.0 * f * W1).T).astype(ml_dtypes.float8_e4m3fn)
    wt2 = np.ascontiguousarray((32.0 * f * W2).T).astype(ml_dtypes.float8_e4m3fn)
    fb = (f * (b1 + b2) / Y_SCALE).astype(np.float32).reshape(1, D)

    ident = np.eye(P, dtype=np.float32)
    return [
        {
            "x": xb[b], "x2": x2b[b], "x3": x3b[b],
            "wt1": wt1, "wt2": wt2, "fb": fb, "ident": ident,
        }
        for b in range(B)
    ]


_NC = None


def kernel(**inputs) -> np.ndarray:
    global _NC
    if _NC is None:
        _NC = build_nc()
    in_maps = prep_inputs(inputs)
    res = run_bass_kernel_spmd(_NC, in_maps, list(range(B)))
    return np.stack(
        [res.results[b]["out"].astype(np.float32) for b in range(B)], axis=0
    )


# revision 33
# speedup vs baseline: 1.1708x; 1.1708x over previous
"""Trainium2 Bass kernel for nn_Cross_Attention (B=8, N=2048, D=768).

Math (per batch b):
    A   = softmax(t, axis=-1) = E/R     (E = exp(t), R = rowsum)
    Q   = softmax(t, axis=0)  = E/S     (S = colsum)
    attn = (x @ A^T) @ Q = x @ KQ,   KQ[d,d'] = (sum_m E[m,d] E[m,d']/R[m]) / S[d']
    out = x @ Msum + fb + x
    Msum = f*(KQ_1 @ W1^T + KQ_2 @ W2^T),  fb = f*(b1 + b2),  f = sigmoid(w)

All heavy matmuls are fp8e4 with MatmulPerfMode.DoubleRow (2 k-tiles
per instruction, 2x PE rate).  The 1/R row normalization rides the
*stationary* operand only (sc = E * C1/R), so KQ Gram matmuls for a
token pair depend only on that pair's exp + rowsum.  The 1/S column
normalization is folded into the *weights* (wts *= 4096/S, in place),
which makes kqt = KQraw^T / 16 symmetric with a constant copy-out
scale - no per-column rescale, no symmetric-fill pass.

Schedule:
  - x2/x3/x arrive as bf16 (host-cast): halves input HBM traffic.
  - x^T comes from 6 XBAR DMA-transposes (one per 128-feature block,
    [2048,128] DRAM -> [128,2048] SBUF), then one fp8 cast per block:
    zero PE/copy cost for transposition.
  - streamed per 2-tile pair (per t): exp (ACT, accum R) -> 1/R (DVE)
    -> sc (gpsimd) -> colsum + KQ-dp0 row (PE, DoubleRow).
  - KQ dp1-5 full rows don't fit PSUM next to both colsum accumulators:
    they run as two PE bursts per t right after that t's stream,
    overlapping the other t's stream / B / C.
  - B (Msum) uses the S-scaled weights; C streams y = xt8 @ msum
    (+ fb via a tiny bf16 ones-matmul) and adds the exact bf16 residual.

Scales: g=E; sc=E*1024/R; kqt=KQraw^T/16; wts=32*f*W^T*4096/S;
msum=(sum kqt*wts)/8 = 1024*Msum; xt8=8*x^T;
y_ps = 8192*(x@Msum+fb); out = y_ps/8192 + x.
"""

import numpy as np
import ml_dtypes

import concourse.bass as bass
import concourse.tile as tile
from concourse import bacc
from concourse import mybir
from concourse.bass_utils import run_bass_kernel_spmd

F32 = mybir.dt.float32
BF16 = mybir.dt.bfloat16
FP8 = mybir.dt.float8e4
DR = mybir.MatmulPerfMode.DoubleRow
MUL = mybir.AluOpType.mult

B = 8
P = 128
D = 768
DT = D // P
C1 = 1024.0
CS = 1.0 / 16.0       # kqt = CS * KQraw^T;  copy scale = CS/C1
CSW = 4096.0          # wts *= CSW/S
CM_DIV = 8.0
CY = 8.0
Y_SCALE = 1.0 / 8192.0

CHUNKS = ((0, 512), (512, 256))

# ---- PSUM map (f32 offsets; banks are 512 f32) ----
# stream: dp0[t0]@0, s[t0]@1024, dp0[t1]@2048, s[t1]@3072, tp@3840/3968
# post bursts reuse freed stream regions (see POST_BURSTS)
# B: m_ps@0/1024;  C: y_ps@0/1024
DP0_OFF = (0, 2048)
S_OFF = (1024, 3072)
TP_OFF = (3840, 3968)
# per burst: list of (dp, ((col_off, width, psum_off), ...))
POST_BURSTS = (
    ((1, ((0, 512, 0), (512, 256, 512))),
     (2, ((0, 512, 1024), (512, 256, 768))),
     (3, ((0, 512, 1536), (512, 256, 1792)))),
    ((4, ((0, 512, 0), (512, 256, 512))),
     (5, ((0, 512, 1024), (512, 256, 768)))),
)


def build_nc(NT=16):
    N = NT * P
    NP = NT // 2
    nc = bacc.Bacc()

    x_d = nc.dram_tensor("x", [N, D], BF16, kind="ExternalInput")
    x2_d = nc.dram_tensor("x2", [N, D], BF16, kind="ExternalInput")
    x3_d = nc.dram_tensor("x3", [N, D], BF16, kind="ExternalInput")
    wt1_d = nc.dram_tensor("wt1", [D, D], FP8, kind="ExternalInput")
    wt2_d = nc.dram_tensor("wt2", [D, D], FP8, kind="ExternalInput")
    fb_d = nc.dram_tensor("fb", [1, D], F32, kind="ExternalInput")
    id_d = nc.dram_tensor("ident", [P, P], F32, kind="ExternalInput")
    out_d = nc.dram_tensor("out", [N, D], BF16, kind="ExternalOutput")

    x2_pr = x2_d.rearrange("(q t p) d -> q p t d", t=2, p=P)
    x3_pr = x3_d.rearrange("(q t p) d -> q p t d", t=2, p=P)
    x_half = x_d.rearrange("(c t p) d -> c p t d", t=8, p=P)
    att_pr = [x2_pr, x3_pr]
    out_t = out_d.rearrange("(t p) d -> t p d", p=P)

    with tile.TileContext(nc) as tc:
        with (
            tc.tile_pool(name="ps", bufs=1, space="PSUM") as psp,
            tc.tile_pool(name="consts", bufs=1) as consts,
            tc.tile_pool(name="big", bufs=1) as big,
            tc.tile_pool(name="stream", bufs=2) as stream,
            tc.tile_pool(name="stats", bufs=2) as stats,
            tc.tile_pool(name="outp", bufs=3) as outp,
        ):
            psb = psp.tile([P, 4096], F32)

            ones8 = consts.tile([P, 2, P], FP8)
            nc.vector.memset(ones8, 1.0)
            onesfb = consts.tile([P, P], BF16)
            nc.vector.memset(onesfb, 1.0 / 128.0)
            ident = consts.tile([P, P], F32)
            fbb = consts.tile([P, D], F32)
            fbby = consts.tile([P, D], BF16)
            wts = consts.tile([P, 2, DT, D], FP8)
            kqt = [
                consts.tile([P, DT, D], FP8, tag=f"kqt{t}", name=f"kqt{t}")
                for t in range(2)
            ]
            msum = consts.tile([P, DT, D], FP8)
            rsw = consts.tile([P, 2, DT], F32)  # CSW/S per d'
            xbig = consts.tile([P, NT, D], BF16)
            xtb = consts.tile([P, DT, N], BF16)  # x^T via XBAR
            xt8 = consts.tile([P, DT, N], FP8)   # 8*x^T
            gate = consts.tile([P, 1], BF16)
            g = [
                big.tile([P, NT, D], FP8, tag=f"g{t}", name=f"g{t}")
                for t in range(2)
            ]
            sc = [
                big.tile([P, NT, D], FP8, tag=f"sc{t}", name=f"sc{t}")
                for t in range(2)
            ]
            xsrc = [
                big.tile([P, NT, D], BF16, tag=f"xs{t}", name=f"xs{t}")
                for t in range(2)
            ]

            rvec = stats.tile([P, 2, NT], F32, tag="rvec")
            rrec = stats.tile([P, 2, NT], F32, tag="rrec")

            def stream_t(t):
                """exp + 1/R + sc + colsum + KQ dp0 row, per pair.

                Input pair DMAs are issued inside the loop (even pairs from
                the idle sync engine, odd pairs from the scalar engine right
                between exps) so HWDGE ring backpressure never stalls the
                scalar engine ahead of compute.
                """
                for q in range(NP):
                    eng = nc.sync if q % 2 == 0 else nc.scalar
                    eng.dma_start(out=xsrc[t][:, 2 * q : 2 * q + 2, :],
                                  in_=att_pr[t][q])
                    if t == 0 and q == 0:
                        # gpsimd consts, gated behind the first input pair
                        nc.gpsimd.dma_start(out=gate, in_=xsrc[0][:, 0, 0:1])
                        for tw, wd in enumerate((wt1_d, wt2_d)):
                            nc.gpsimd.dma_start(
                                out=wts[:, tw],
                                in_=wd.rearrange("(c p) j -> p c j", p=P),
                            )
                        nc.gpsimd.dma_start(out=ident, in_=id_d[:, :])
                        nc.gpsimd.dma_start(
                            out=fbb, in_=fb_d[0:1, :].to_broadcast([P, D])
                        )
                        nc.vector.tensor_copy(fbby, fbb)
                    if t == 1 and q >= 6:
                        # x residual halves ride the scalar ring after x3-odd
                        nc.scalar.dma_start(
                            out=xbig[:, 8 * (q - 6) : 8 * (q - 6) + 8, :],
                            in_=x_half[q - 6],
                        )
                    for j in range(2):
                        i = 2 * q + j
                        nc.scalar.activation(
                            out=g[t][:, i, :], in_=xsrc[t][:, i, :],
                            func=mybir.ActivationFunctionType.Exp,
                            accum_out=rvec[:, t, i : i + 1],
                        )
                    nc.vector.reciprocal(rrec[:, t, 2 * q : 2 * q + 2],
                                         rvec[:, t, 2 * q : 2 * q + 2])
                    for j in range(2):
                        i = 2 * q + j
                        nc.gpsimd.tensor_scalar(
                            out=sc[t][:, i, :], in0=g[t][:, i, :],
                            scalar1=rrec[:, t, i : i + 1], scalar2=C1,
                            op0=MUL, op1=MUL,
                        )
                    gpr = g[t][:, 2 * q : 2 * q + 2, :]
                    spr = sc[t][:, 2 * q : 2 * q + 2, :]
                    s_ps = psb[:, S_OFF[t] : S_OFF[t] + D]
                    d0 = psb[:, DP0_OFF[t] : DP0_OFF[t] + D]
                    for off, sz in CHUNKS:
                        nc.tensor.matmul(
                            s_ps[:, off : off + sz], ones8,
                            gpr[:, :, off : off + sz],
                            start=(q == 0), stop=(q == NP - 1), perf_mode=DR,
                        )
                    for off, sz in CHUNKS:
                        nc.tensor.matmul(
                            d0[:, off : off + sz], spr[:, :, 0:P],
                            gpr[:, :, off : off + sz],
                            start=(q == 0), stop=(q == NP - 1), perf_mode=DR,
                        )

            def epilogue_t(t):
                """dp0 copy, 4096/S, wts scaling, dp1-5 bursts."""
                nc.vector.tensor_scalar_mul(
                    kqt[t][:, 0, :], psb[:, DP0_OFF[t] : DP0_OFF[t] + D],
                    CS / C1,
                )
                s_ps = psb[:, S_OFF[t] : S_OFF[t] + D]
                rsb = stream.tile([P, D], F32, tag="rsb", bufs=2)
                nc.vector.reciprocal(rsb, s_ps)
                nc.vector.tensor_scalar_mul(rsb, rsb, CSW)
                for c in range(DT):
                    tp = psb[:, TP_OFF[c % 2] : TP_OFF[c % 2] + P]
                    nc.tensor.transpose(tp, rsb[:, c * P : (c + 1) * P], ident)
                    nc.vector.tensor_copy(rsw[:, t, c : c + 1], tp[:, 0:1])
                for dp in range(DT):
                    # fold 1/S into the weights (in place, per d' row)
                    nc.vector.tensor_scalar_mul(
                        wts[:, t, dp, :], wts[:, t, dp, :],
                        rsw[:, t, dp : dp + 1],
                    )
                for burst in POST_BURSTS:
                    for dp, chunks in burst:
                        lhsT_all = sc[t]
                        for off, sz, poff in chunks:
                            for q in range(NP):
                                nc.tensor.matmul(
                                    psb[:, poff : poff + sz],
                                    sc[t][:, 2 * q : 2 * q + 2,
                                          dp * P : (dp + 1) * P],
                                    g[t][:, 2 * q : 2 * q + 2, off : off + sz],
                                    start=(q == 0), stop=(q == NP - 1),
                                    perf_mode=DR,
                                )
                    for dp, chunks in burst:
                        for off, sz, poff in chunks:
                            nc.vector.tensor_scalar_mul(
                                kqt[t][:, dp, off : off + sz],
                                psb[:, poff : poff + sz],
                                CS / C1,
                            )

            stream_t(0)
            epilogue_t(0)
            stream_t(1)
            # x^T XBAR transposes ride the sync ring behind x3-even
            for c in range(DT):
                nc.sync.dma_start_transpose(
                    xtb[:, c, :], x_d[:, c * P : (c + 1) * P]
                )
            epilogue_t(1)
            # 8*x^T fp8 cast
            for c in range(DT):
                nc.vector.tensor_scalar_mul(xt8[:, c, :], xtb[:, c, :], CY)

            # ---- Msum ----
            for d in range(DT):
                mb = 1024 * (d % 2)
                m_ps = psb[:, mb : mb + D]
                for t in range(2):
                    for dpp in range(0, DT, 2):
                        lhsT = kqt[t][:, dpp : dpp + 2, d * P : (d + 1) * P]
                        for off, sz in CHUNKS:
                            nc.tensor.matmul(
                                m_ps[:, off : off + sz], lhsT,
                                wts[:, t, dpp : dpp + 2, off : off + sz],
                                start=(t == 0 and dpp == 0),
                                stop=(t == 1 and dpp == DT - 2),
                                perf_mode=DR,
                            )
                if d % 2 == 0:
                    nc.vector.tensor_scalar_mul(msum[:, d, :], m_ps, 1.0 / CM_DIV)
                else:
                    nc.scalar.mul(msum[:, d, :], m_ps, 1.0 / CM_DIV)

            # ---- y phase ----
            for i in range(NT):
                yb = 1024 * (i % 2)
                y_ps = psb[:, yb : yb + D]
                for off, sz in CHUNKS:
                    nc.tensor.matmul(
                        y_ps[:, off : off + sz], onesfb, fbby[:, off : off + sz],
                        start=True, stop=False,
                    )
                for k in range(0, DT, 2):
                    for off, sz in CHUNKS:
                        nc.tensor.matmul(
                            y_ps[:, off : off + sz],
                            xt8[:, k : k + 2, i * P : (i + 1) * P],
                            msum[:, k : k + 2, off : off + sz],
                            start=False, stop=(k == DT - 2),
                            perf_mode=DR,
                        )
                oi = outp.tile([P, D], F32, tag="out")
                nc.vector.scalar_tensor_tensor(
                    out=oi, in0=y_ps, scalar=Y_SCALE, in1=xbig[:, i, :],
                    op0=MUL, op1=mybir.AluOpType.add,
                )
                nc.gpsimd.dma_start(out=out_t[i], in_=oi)

    nc.compile()
    return nc


def prep_inputs(inputs):
    x = np.asarray(inputs["x"], dtype=np.float32)
    x2 = np.asarray(inputs["x2"], dtype=np.float32)
    x3 = np.asarray(inputs["x3"], dtype=np.float32)
    W1 = np.asarray(inputs["W1"], dtype=np.float32)
    b1 = np.asarray(inputs["b1"], dtype=np.float32)
    W2 = np.asarray(inputs["W2"], dtype=np.float32)
    b2 = np.asarray(inputs["b2"], dtype=np.float32)
    w = np.asarray(inputs["w"], dtype=np.float32)

    xb = np.ascontiguousarray(x).astype(ml_dtypes.bfloat16)
    x2b = np.ascontiguousarray(x2).astype(ml_dtypes.bfloat16)
    x3b = np.ascontiguousarray(x3).astype(ml_dtypes.bfloat16)

    f = 1.0 / (1.0 + np.exp(-float(w.reshape(-1)[0])))
    wt1 = np.ascontiguousarray((32.0 * f * W1).T).astype(ml_dtypes.float8_e4m3fn)
    wt2 = np.ascontiguousarray((32.0 * f * W2).T).astype(ml_dtypes.float8_e4m3fn)
    fb = (f * (b1 + b2) / Y_SCALE).astype(np.float32).reshape(1, D)

    ident = np.eye(P, dtype=np.float32)
    return [
        {
            "x": xb[b], "x2": x2b[b], "x3": x3b[b],
            "wt1": wt1, "wt2": wt2, "fb": fb, "ident": ident,
        }
        for b in range(B)
    ]


_NC = None


def kernel(**inputs) -> np.ndarray:
    global _NC
    if _NC is None:
        _NC = build_nc()
    in_maps = prep_inputs(inputs)
    res = run_bass_kernel_spmd(_NC, in_maps, list(range(B)))
    return np.stack(
        [res.results[b]["out"].astype(np.float32) for b in range(B)], axis=0
    )
